# revision 22
# baseline (speedup 1.0000x reference)
"""Trainium2 Bass kernel for causal multi-head attention.

Problem shapes (hardcoded):
  x: [B=2, S=2048, D_MODEL=768] f32
  W_Q/W_K/W_V: [H=12, 768, 64], W_O: [12, 64, 768], biases b_Q/b_K/b_V: [12,64], b_O: [768]
  out: [2, 2048, 768] f32

Sharding: 8 cores; core c owns batch b = c // 4 and heads [3*(c%4), 3*(c%4)+3).
Each core computes a partial output sum over its 3 heads; the host sums the
4 partials per batch (the "all-reduce" of the output projection).

Per-core formulation (everything bf16 into the PE, fp32 PSUM accumulate):
  xT_aug = [x[b].T; ones]                           [769, 2048]   (host)
  Q^T/K^T[h] = W_aug[h].T @ xT_aug                  [64, 2048]    (PE, M-packed pairs)
  V[h] = (xT_aug chunks).T @ Wv_aug                 [2048, 64+1s] (PE, natural layout)
  scores^T[kb, qt] = K^T_blk.T @ Q^T                [128, 512]    (PE, head pair row-packed)
  expT = exp(0.125 * scores^T)                      (ACT, PSUM->SBUF bf16)
  expT *= causal 0/1 mask on diagonal tiles          (DVE)
  z'^T[h,qt] += V'_blk.T @ expT  (V' has ones col => row 64 = softmax denom)
  zT = z'[0:64] * (1/denom broadcast)               (DVE + DMA bcast)
  out[st] = sum_h zT_h_blk.T @ W_O_h + b_O          (PE + DVE add)
"""

import sys
import types

for _p in ("/opt/trn_rl_repo",):
    if _p not in sys.path:
        sys.path.insert(0, _p)

import numpy as np
import ml_dtypes

BF16 = ml_dtypes.bfloat16

B, S, D_MODEL, N_HEADS, D_HEAD = 2, 2048, 768, 12, 64
N_CORES = 8
HEADS_PER_CORE = 3
SCALE = 1.0 / 8.0  # 1/sqrt(d_head)

_CACHE = {}


def _ensure_ntff_hook():
    """Register the axon NTFF profile hook if the image lacks antenv.axon_hooks."""
    try:
        import antenv.axon_hooks  # noqa: F401
        return
    except ImportError:
        pass
    import antenv
    mod = types.ModuleType("antenv.axon_hooks")
    _h = [None]
    mod.set_axon_ntff_profile_hook = lambda h: _h.__setitem__(0, h)
    mod.get_axon_ntff_profile_hook = lambda: _h[0]
    sys.modules["antenv.axon_hooks"] = mod
    antenv.axon_hooks = mod
    try:
        from trn_agent_boot.trn_boot import _ntff_profile_via_ctypes
        hook = _ntff_profile_via_ctypes("/opt/axon/libaxon_pjrt.so")
        if hook is not None:
            mod.set_axon_ntff_profile_hook(hook)
    except Exception:
        pass


def build_bass(debug_dumps=False):
    """Build and compile the per-core Bass program (same NEFF on all 8 cores)."""
    key = ("nc", debug_dumps)
    if key in _CACHE:
        return _CACHE[key]

    import concourse.bass as bass
    import concourse.mybir as mybir
    import concourse.tile as tile
    from concourse import bacc
    from concourse.bass import ts

    f32 = mybir.dt.float32
    bf16 = mybir.dt.bfloat16
    Exp = mybir.ActivationFunctionType.Exp
    Log = mybir.ActivationFunctionType.Ln

    nc = bacc.Bacc("TRN2", target_bir_lowering=False, debug=False, num_devices=N_CORES)

    # DRAM I/O (per-core)
    xt_d = nc.dram_tensor("xt", [769, 2048], bf16, kind="ExternalInput").ap()
    wqk_d = nc.dram_tensor("wqk", [769, 384], bf16, kind="ExternalInput").ap()
    wv_d = nc.dram_tensor("wv", [768, 192], bf16, kind="ExternalInput").ap()
    wo_d = nc.dram_tensor("wo", [192, 768], bf16, kind="ExternalInput").ap()
    bo_d = nc.dram_tensor("bo", [128, 768], f32, kind="ExternalInput").ap()
    mask_d = nc.dram_tensor("mask", [128, 128], bf16, kind="ExternalInput").ap()
    bqk_d = nc.dram_tensor("bqk", [128, 3], f32, kind="ExternalInput").ap()
    out_d = nc.dram_tensor("out", [2048, 768], f32, kind="ExternalOutput").ap()
    dbg = {}
    if debug_dumps:
        dbg["qt01"] = nc.dram_tensor("dQT01", [128, 2048], bf16, kind="ExternalOutput").ap()
        dbg["vp"] = nc.dram_tensor("dVp", [128, 3120], bf16, kind="ExternalOutput").ap()
        dbg["zt0"] = nc.dram_tensor("dZT0", [64, 2048], bf16, kind="ExternalOutput").ap()
        dbg["rbc"] = nc.dram_tensor("dRBC", [64, 512], f32, kind="ExternalOutput").ap()
        dbg["ex"] = nc.dram_tensor("dEX", [128, 1024], bf16, kind="ExternalOutput").ap()
        dbg["den"] = nc.dram_tensor("dDEN", [1, 512], f32, kind="ExternalOutput").ap()

    with tile.TileContext(nc) as tc:
        from contextlib import ExitStack

        with ExitStack() as ctx:
            setup = ctx.enter_context(tc.tile_pool(name="setup", bufs=1))

            # ---- PE warmup: ~4us of back-to-back matmuls while DMAs load.
            # Flips the HAM clock gate to 8/8 (2.4 GHz) before real work.
            with tc.tile_pool(name="warm_ps", bufs=1, space="PSUM") as warm_ps:
                wsrc = setup.tile([128, 512], bf16, tag="wsrc")
                nc.vector.memset(wsrc[:], 0.0)
                wps = warm_ps.tile([128, 512], f32, tag="wps")
                for _ in range(28):
                    nc.tensor.matmul(wps[:, :], lhsT=wsrc[:, 0:128], rhs=wsrc[:, :],
                                     start=True, stop=True)
                # consume so nothing downstream DCEs the burst
                wout = setup.tile([1, 4], f32, tag="wout")
                nc.vector.tensor_copy(wout[:], wps[0:1, 0:4])


            # ---- load inputs to SBUF ----
            xt = []  # 6x [128, 2048] + 1x [1, 2048] bf16
            for mc in range(6):
                t = setup.tile([128, 2048], bf16, tag=f"xt{mc}")
                nc.sync.dma_start(t[:], xt_d[ts(mc, 128), :])
                xt.append(t)
            t = setup.tile([1, 2048], bf16, tag="xt6")
            nc.sync.dma_start(t[:], xt_d[768:769, :])
            xt.append(t)

            wqk = []
            for mc in range(6):
                t = setup.tile([128, 384], bf16, tag=f"wqk{mc}")
                nc.sync.dma_start(t[:], wqk_d[ts(mc, 128), :])
                wqk.append(t)
            t = setup.tile([1, 384], bf16, tag="wqk6")
            nc.sync.dma_start(t[:], wqk_d[768:769, :])
            wqk.append(t)

            wv = []
            for mc in range(6):
                t = setup.tile([128, 192], bf16, tag=f"wv{mc}")
                nc.sync.dma_start(t[:], wv_d[ts(mc, 128), :])
                wv.append(t)

            wo01 = setup.tile([128, 768], bf16, tag="wo01")
            nc.sync.dma_start(wo01[:], wo_d[0:128, :])
            wo2 = setup.tile([64, 768], bf16, tag="wo2")
            nc.sync.dma_start(wo2[:], wo_d[128:192, :])

            bo_bc = setup.tile([128, 768], f32, tag="bo")
            nc.sync.dma_start(bo_bc[:], bo_d[:, :])
            bqk = setup.tile([128, 3], f32, tag="bqk")
            nc.sync.dma_start(bqk[:], bqk_d[:, :])

            # ---- causal 0/1 triangle mask [128, 128] (host-generated) ----
            mask = setup.tile([128, 128], bf16, tag="mask")
            nc.sync.dma_start(mask[:], mask_d[:, :])

            # persistent activation storage
            QT01 = setup.tile([128, 2048], bf16, tag="QT01")  # heads 0,1 (d on parts)
            KT01 = setup.tile([128, 2048], bf16, tag="KT01")
            Q2T = setup.tile([64, 2048], bf16, tag="Q2T")
            K2T = setup.tile([64, 2048], bf16, tag="K2T")
            Q2Tb = setup.tile([128, 2048], bf16, tag="Q2Tb")  # rows 64:128 used
            K2Tb = setup.tile([128, 2048], bf16, tag="K2Tb")
            Vp = setup.tile([128, 3120], bf16, tag="Vp")  # [s128, (h, sb, d+1)]
            Vp_r = Vp[:].rearrange("p (h s d) -> p h s d", h=3, s=16)
            zT01 = setup.tile([128, 2048], bf16, tag="zT01")  # h0 rows 0:64, h1 64:128
            zT2 = setup.tile([64, 2048], bf16, tag="zT2")

            # ================= projections =================
            kshift = ctx.enter_context(tc.tile_pool(name="kshift", bufs=2))
            with tc.tile_pool(name="proj_ps", bufs=3, space="PSUM") as proj_ps:
                # Q^T/K^T: 3 M-passes: [Qh0|Qh1], [Kh0|Kh1], [Qh2|Kh2]
                for pi, c0 in enumerate((0, 128, 256)):
                    for st in range(4):
                        ps = proj_ps.tile([128, 512], f32, tag="qkps")
                        for mc in range(6):
                            nc.tensor.matmul(
                                ps[:, :],
                                lhsT=wqk[mc][:, c0:c0 + 128],
                                rhs=xt[mc][:, ts(st, 512)],
                                start=(mc == 0),
                                stop=(mc == 5),
                            )
                        # psum->SBUF copy folds in the per-partition bias
                        if pi == 0:
                            nc.vector.tensor_scalar_add(
                                QT01[:, ts(st, 512)], ps[:, :], bqk[:, 0:1])
                        elif pi == 1:
                            nc.vector.tensor_scalar_add(
                                KT01[:, ts(st, 512)], ps[:, :], bqk[:, 1:2])
                        else:
                            nc.scalar.activation(
                                Q2T[:, ts(st, 512)], ps[0:64, :],
                                mybir.ActivationFunctionType.Identity,
                                bias=bqk[0:64, 2:3])
                            ksh = kshift.tile([128, 512], bf16, tag="ksh",
                                              name=f"ksh{st}")
                            nc.vector.tensor_scalar_add(
                                ksh[64:128, :], ps[64:128, :], bqk[64:128, 2:3])
                            nc.sync.dma_start(K2T[:, ts(st, 512)], ksh[64:128, :])
                            nc.sync.dma_start(K2Tb[64:128, ts(st, 512)], ksh[64:128, :])
                            nc.sync.dma_start(Q2Tb[64:128, ts(st, 512)], Q2T[:, ts(st, 512)])

                # V natural layout: per 128-row s-block
                nc.vector.memset(Vp_r[:, :, :, 64:65], 1.0)  # denom ones cols
                for sb in range(16):
                    ps = proj_ps.tile([128, 512], f32, tag="vps")
                    o = ps[:, 0:192]
                    for mc in range(6):
                        nc.tensor.matmul(
                            o,
                            lhsT=xt[mc][:, ts(sb, 128)],
                            rhs=wv[mc][:, :],
                            start=(mc == 0),
                            stop=(mc == 5),
                        )
                    # strided copy [128, 3, 64] <- [128, 3, 64]
                    nc.vector.tensor_copy(
                        Vp_r[:, :, sb, 0:64],
                        ps[:, 0:192].rearrange("p (h d) -> p h d", h=3),
                    )

            # ================= attention =================
            # zs: fp32 staging of z' (incl denom row 64); normalization is
            # deferred + batched to avoid ACT table-set thrashing (Ln/Exp).
            zs = [setup.tile([65, 2048], f32, tag=f"zs{h}", name=f"zs{h}") for h in range(3)]

            nrm = ctx.enter_context(tc.tile_pool(name="nrm", bufs=4))
            nrm_dram = ctx.enter_context(tc.tile_pool(name="nrm_dram", bufs=2, space="DRAM"))

            def normalize(hlist):
                """zT[h] = zs[h][0:64] / zs[h][64] for all qt, batched per pass."""
                n = len(hlist) * 4
                den_all = nrm.tile([n, 512], f32, tag="den_all",
                                   name=f"den_all{hlist[0]}")
                for i, h in enumerate(hlist):
                    for qt in range(4):
                        nc.sync.dma_start(
                            den_all[i * 4 + qt:i * 4 + qt + 1, :],
                            zs[h][64:65, ts(qt, 512)],
                        )
                ln_all = nrm.tile([n, 512], f32, tag="ln_all", name=f"ln{hlist[0]}")
                nc.scalar.activation(ln_all[:], den_all[:], Log)
                rec_all = nrm.tile([n, 512], f32, tag="rec_all", name=f"rec{hlist[0]}")
                nc.scalar.activation(rec_all[:], ln_all[:], Exp, scale=-1.0)
                dsc = nrm_dram.tile([n, 512], f32, tag="dsc", name=f"dsc{hlist[0]}")
                nc.sync.dma_start(dsc[:], rec_all[:])
                for i, h in enumerate(hlist):
                    for qt in range(4):
                        u = i * 4 + qt
                        rbc = nrm.tile([64, 512], f32, tag="rbc", name=f"rbc{h}_{qt}")
                        nc.sync.dma_start(
                            rbc[:], dsc[u:u + 1, :].broadcast_to((64, 512))
                        )
                        if h == 0:
                            nc.vector.tensor_mul(
                                zT01[0:64, ts(qt, 512)], zs[h][0:64, ts(qt, 512)], rbc[:]
                            )
                        elif h == 2:
                            nc.vector.tensor_mul(
                                zT2[:, ts(qt, 512)], zs[h][0:64, ts(qt, 512)], rbc[:]
                            )
                        else:
                            # h1 lands on partitions 64:128 of zT01: DVE is
                            # partition-locked, so mul to a temp then DMA-shift
                            zsh = nrm.tile([64, 512], bf16, tag="zsh",
                                           name=f"zsh{qt}")
                            nc.vector.tensor_mul(
                                zsh[:], zs[h][0:64, ts(qt, 512)], rbc[:]
                            )
                            nc.sync.dma_start(zT01[64:128, ts(qt, 512)], zsh[:])

            with (
                tc.tile_pool(name="sc_ps", bufs=2, space="PSUM") as sc_ps,
                tc.tile_pool(name="zp_ps", bufs=1, space="PSUM") as zp_ps,
                tc.tile_pool(name="out_ps", bufs=1, space="PSUM") as out_ps,
                tc.tile_pool(name="expp", bufs=4) as expp,
                tc.tile_pool(name="outp", bufs=3) as outp,
            ):
                def outproj(st):
                    ps = out_ps.tile([128, 768], f32, tag="ops", name=f"ops{st}")
                    for n0, nw in ((0, 512), (512, 256)):
                        nc.tensor.matmul(
                            ps[:, n0:n0 + nw],
                            lhsT=zT01[:, ts(st, 128)],
                            rhs=wo01[:, n0:n0 + nw],
                            start=True, stop=False,
                        )
                        nc.tensor.matmul(
                            ps[:, n0:n0 + nw],
                            lhsT=zT2[:, ts(st, 128)],
                            rhs=wo2[:, n0:n0 + nw],
                            start=False, stop=True,
                        )
                    ot = outp.tile([128, 768], f32, tag="ot", name=f"ot{st}")
                    nc.vector.tensor_add(ot[:], ps[:, :], bo_bc[:])
                    nc.sync.dma_start(out_d[ts(st, 128), :], ot[:])

                def attend(heads, on_qt_done=None):
                    """heads: (h, kt_fn(kb, j), qt_fn(qt, j, qc0)); j = kb parity
                    (lets pass B alternate base-0/base-64 tiles for pairing).
                    Diagonal tiles skip fully-masked columns entirely."""
                    for qt in range(4):
                        nkb = 4 * qt + 4
                        zp = {}
                        for h, _, _ in heads:
                            zp[h] = zp_ps.tile([65, 512], f32, tag=f"zp{h % 2}", name=f"zp{h}")
                        for g in range(nkb // 2):
                            kbs = (2 * g, 2 * g + 1)
                            qc0s = [max(0, 128 * (kb - 4 * qt)) for kb in kbs]
                            sc = {}
                            for h, _, _ in heads:
                                sc[h] = sc_ps.tile([128, 1024], f32, tag="sc", name=f"sc{h}")
                            for j, kb in enumerate(kbs):
                                for h, kt_fn, qt_fn in heads:
                                    nc.tensor.matmul(
                                        sc[h][:, 512 * j + qc0s[j]:512 * (j + 1)],
                                        lhsT=kt_fn(kb, j),
                                        rhs=qt_fn(qt, j, qc0s[j]),
                                        start=True,
                                        stop=True,
                                    )
                            diag = kbs[0] >= 4 * qt
                            ex = {}
                            for h, _, _ in heads:
                                e = expp.tile([128, 1024], bf16, tag="ex", name=f"ex{h}")
                                if not diag:
                                    nc.scalar.activation(e[:], sc[h][:], Exp, scale=SCALE)
                                else:
                                    for j in range(2):
                                        s0 = 512 * j + qc0s[j]
                                        nc.scalar.activation(
                                            e[:, s0:512 * (j + 1)],
                                            sc[h][:, s0:512 * (j + 1)],
                                            Exp, scale=SCALE)
                                        # triangular 128-col block gets the 0/1 mask
                                        nc.vector.tensor_mul(
                                            e[:, s0:s0 + 128], e[:, s0:s0 + 128], mask[:]
                                        )
                                ex[h] = e
                            for j, kb in enumerate(kbs):
                                for h, _, _ in heads:
                                    nc.tensor.matmul(
                                        zp[h][:, qc0s[j]:512],
                                        lhsT=Vp_r[:, h, kb, :],
                                        rhs=ex[h][:, 512 * j + qc0s[j]:512 * (j + 1)],
                                        start=(kb == 0),
                                        stop=(kb == nkb - 1),
                                        skip_group_check=True,
                                    )
                        for h, _, _ in heads:
                            # stage z' (+denom) to SBUF fp32; frees the psum
                            nc.scalar.copy(zs[h][:, ts(qt, 512)], zp[h][:, :])
                        if on_qt_done is not None:
                            on_qt_done(qt)

                # pass A: heads 0,1 row-packed (concurrent PE row groups)
                attend(
                    [
                        (0, lambda kb, j: KT01[0:64, ts(kb, 128)],
                         lambda q, j, c0: QT01[0:64, q * 512 + c0:(q + 1) * 512]),
                        (1, lambda kb, j: KT01[64:128, ts(kb, 128)],
                         lambda q, j, c0: QT01[64:128, q * 512 + c0:(q + 1) * 512]),
                    ]
                )
                normalize([0, 1])  # overlaps pass B on PE

                def h2_finish(qt):
                    """Exact DVE reciprocal (no ACT table switch), broadcast,
                    normalize zT2, then the 4 output-projection tiles of qt —
                    all interleaved into pass B's PE stream."""
                    rec = nrm.tile([1, 512], f32, tag="rec2", name=f"rec2_{qt}")
                    nc.vector.reciprocal(rec[:], zs[2][64:65, ts(qt, 512)])
                    dsc = nrm_dram.tile([1, 512], f32, tag="dsc2", name=f"dsc2_{qt}")
                    nc.sync.dma_start(dsc[:], rec[:])
                    rbc = nrm.tile([64, 512], f32, tag="rbc", name=f"rbc2_{qt}")
                    nc.sync.dma_start(rbc[:], dsc[:].broadcast_to((64, 512)))
                    nc.vector.tensor_mul(
                        zT2[:, ts(qt, 512)], zs[2][0:64, ts(qt, 512)], rbc[:]
                    )
                    for st in range(4 * qt, 4 * qt + 4):
                        outproj(st)

                # pass B: head 2, kb-parity pairing via base-64 duplicates
                attend(
                    [
                        (2, lambda kb, j: K2T[:, ts(kb, 128)] if j == 0
                            else K2Tb[64:128, ts(kb, 128)],
                         lambda q, j, c0: Q2T[:, q * 512 + c0:(q + 1) * 512] if j == 0
                            else Q2Tb[64:128, q * 512 + c0:(q + 1) * 512]),
                    ],
                    on_qt_done=h2_finish,
                )

    nc.compile()
    _CACHE[key] = nc
    return nc


def _prep_core_inputs(c, x, W_Q, W_K, W_V, b_Q, b_K, b_V, W_O, b_O):
    b = c // 4
    h0 = HEADS_PER_CORE * (c % 4)
    hs = [h0, h0 + 1, h0 + 2]

    xt = np.empty((769, 2048), np.float32)
    xt[:768] = x[b].T
    xt[768] = 1.0

    def aug(W, bias):  # [768,64]+[64] -> [769,64]
        return np.concatenate([W, bias[None, :]], axis=0)

    wqk = np.concatenate(
        [
            aug(W_Q[hs[0]], b_Q[hs[0]]), aug(W_Q[hs[1]], b_Q[hs[1]]),
            aug(W_K[hs[0]], b_K[hs[0]]), aug(W_K[hs[1]], b_K[hs[1]]),
            aug(W_Q[hs[2]], b_Q[hs[2]]), aug(W_K[hs[2]], b_K[hs[2]]),
        ],
        axis=1,
    )  # [769, 384]
    wv = np.concatenate([W_V[h] for h in hs], axis=1)  # [768, 192]
    wo = np.concatenate([W_O[h] for h in hs], axis=0)  # [192, 768]
    # b_O added once per batch group (divided by the 4 cores that sum); each
    # core's own b_V contribution is exact since attention rows sum to 1.
    bo_eff = b_O / 4.0 + sum(b_V[h] @ W_O[h] for h in hs)
    bo = np.broadcast_to(bo_eff[None, :], (128, 768)).astype(np.float32).copy()
    bqk = np.stack([
        np.concatenate([b_Q[hs[0]], b_Q[hs[1]]]),
        np.concatenate([b_K[hs[0]], b_K[hs[1]]]),
        np.concatenate([b_Q[hs[2]], b_K[hs[2]]]),
    ], axis=1).astype(np.float32)

    kr = np.arange(128)[:, None]
    cc = np.arange(128)[None, :]
    mask = (cc >= kr)  # [128, 128] causal triangle

    return {
        "xt": xt.astype(BF16),
        "mask": mask.astype(BF16),
        "bqk": bqk,
        "wqk": wqk.astype(BF16),
        "wv": wv.astype(BF16),
        "wo": wo.astype(BF16),
        "bo": bo,
    }


def run_sharded(inputs, trace=False, trace_cores=None):
    """Run the SPMD kernel; returns (out [2,2048,768] f32, BassKernelResults)."""
    _ensure_ntff_hook()
    from concourse.bass_utils import run_bass_kernel_spmd

    nc = build_bass()
    in_maps = [
        _prep_core_inputs(c, inputs["normalized_resid_pre"], inputs["W_Q"],
                          inputs["W_K"], inputs["W_V"], inputs["b_Q"], inputs["b_K"],
                          inputs["b_V"], inputs["W_O"], inputs["b_O"])
        for c in range(N_CORES)
    ]
    kwargs = {}
    if trace:
        kwargs["trace"] = True
        kwargs["trace_cores"] = trace_cores if trace_cores is not None else [0]
    res = run_bass_kernel_spmd(nc, in_maps, core_ids=list(range(N_CORES)), **kwargs)

    out = np.zeros((B, S, D_MODEL), np.float32)
    for c in range(N_CORES):
        out[c // 4] += res.results[c]["out"]
    return out, res


def kernel(normalized_resid_pre, W_Q, W_K, W_V, b_Q, b_K, b_V, W_O, b_O):
    inputs = dict(normalized_resid_pre=np.asarray(normalized_resid_pre, np.float32),
                  W_Q=np.asarray(W_Q, np.float32), W_K=np.asarray(W_K, np.float32),
                  W_V=np.asarray(W_V, np.float32), b_Q=np.asarray(b_Q, np.float32),
                  b_K=np.asarray(b_K, np.float32), b_V=np.asarray(b_V, np.float32),
                  W_O=np.asarray(W_O, np.float32), b_O=np.asarray(b_O, np.float32))
    out, _ = run_sharded(inputs, trace=False)
    return out


# revision 23
# speedup vs baseline: 1.0921x; 1.0921x over previous
"""Trainium2 Bass kernel for causal multi-head attention.

Problem shapes (hardcoded):
  x: [B=2, S=2048, D_MODEL=768] f32
  W_Q/W_K/W_V: [H=12, 768, 64], W_O: [12, 64, 768], b_*: per-head biases
  out: [2, 2048, 768] f32

Sharding: 8 cores; core c owns batch b = c // 4 and heads [3*(c%4), 3*(c%4)+3).
Each core computes a partial output over its 3 heads; the host sums the 4
partials per batch (the "all-reduce" of the output projection).

Per-core dataflow (bf16 matmul operands, fp32 PSUM accumulation):
  xT_aug = [x[b].T; ones]                            [769, 2048]  (host)
  Q^T/K^T = W_pair.T @ xT  (heads packed in M)       [128, 2048]  PE
  V       = xT_blk.T @ Wv  (natural layout)          [2048, 3*64] PE
  scores^T[kb, qt] = K^T_blk.T @ Q^T                 [128, 512]   PE (row-
     group pairing: h0 at partitions 0:64, h1 at 64:128 run concurrently;
     head 2 pairs across kb parity via base-64 duplicate tiles)
  expT = exp(0.125 * scores^T)  (diagonal tiles skip fully-masked columns;
     the 128-wide triangle gets a 0/1 bf16 mask multiply)       ACT+DVE
  z'^T += V'_blk.T @ expT   (V' ones column => row 64 = softmax denom) PE
  zT = z'[0:64] * (1/denom)  (1/x = exp(-ln(x)) on ACT, batched;
     broadcast via DRAM round trip)                  DVE
  out[st] = zT01_blk.T @ WO01 + zT2_blk.T @ WO2 + bo  PE + DVE

All biases are exact: b_Q/b_K ride the PSUM->SBUF copies as per-partition
bias APs; b_V/b_O fold into bo on the host (attention rows sum to 1).
"""

import sys
import types

for _p in ("/opt/trn_rl_repo",):
    if _p not in sys.path:
        sys.path.insert(0, _p)

import numpy as np
import ml_dtypes

BF16 = ml_dtypes.bfloat16

B, S, D_MODEL, N_HEADS, D_HEAD = 2, 2048, 768, 12, 64
N_CORES = 8
HEADS_PER_CORE = 3
SCALE = 1.0 / 8.0  # 1/sqrt(d_head)

_CACHE = {}


def _ensure_ntff_hook():
    """Register the axon NTFF profile hook if the image lacks antenv.axon_hooks."""
    try:
        import antenv.axon_hooks  # noqa: F401
        return
    except ImportError:
        pass
    import antenv
    mod = types.ModuleType("antenv.axon_hooks")
    _h = [None]
    mod.set_axon_ntff_profile_hook = lambda h: _h.__setitem__(0, h)
    mod.get_axon_ntff_profile_hook = lambda: _h[0]
    sys.modules["antenv.axon_hooks"] = mod
    antenv.axon_hooks = mod
    try:
        from trn_agent_boot.trn_boot import _ntff_profile_via_ctypes
        hook = _ntff_profile_via_ctypes("/opt/axon/libaxon_pjrt.so")
        if hook is not None:
            mod.set_axon_ntff_profile_hook(hook)
    except Exception:
        pass


def build_bass():
    """Build and compile the per-core Bass program (same NEFF on all 8 cores)."""
    if "nc" in _CACHE:
        return _CACHE["nc"]

    import concourse.mybir as mybir
    import concourse.tile as tile
    from concourse import bacc
    from concourse.bass import ts, _add_dep_helper
    from contextlib import ExitStack

    f32 = mybir.dt.float32
    bf16 = mybir.dt.bfloat16
    Exp = mybir.ActivationFunctionType.Exp
    Log = mybir.ActivationFunctionType.Ln
    Ident = mybir.ActivationFunctionType.Identity

    nc = bacc.Bacc("TRN2", target_bir_lowering=False, debug=False, num_devices=N_CORES)

    xt_d = nc.dram_tensor("xt", [769, 2048], bf16, kind="ExternalInput").ap()
    wqk_d = nc.dram_tensor("wqk", [769, 384], bf16, kind="ExternalInput").ap()
    wv_d = nc.dram_tensor("wv", [768, 192], bf16, kind="ExternalInput").ap()
    wo_d = nc.dram_tensor("wo", [192, 768], bf16, kind="ExternalInput").ap()
    bo_d = nc.dram_tensor("bo", [128, 768], f32, kind="ExternalInput").ap()
    mask_d = nc.dram_tensor("mask", [128, 128], bf16, kind="ExternalInput").ap()
    bqk_d = nc.dram_tensor("bqk", [128, 3], f32, kind="ExternalInput").ap()
    out_d = nc.dram_tensor("out", [2048, 768], f32, kind="ExternalOutput").ap()

    with tile.TileContext(nc) as tc, ExitStack() as ctx:
        setup = ctx.enter_context(tc.tile_pool(name="setup", bufs=1))

        # ---- PE warmup while input DMAs stream in: flips the HAM clock
        # gate to 8/8 (2.4 GHz) before real work ----
        with tc.tile_pool(name="warm_ps", bufs=1, space="PSUM") as warm_ps:
            wsrc = setup.tile([128, 512], bf16, tag="wsrc")
            nc.vector.memset(wsrc[:], 0.0)
            wps = warm_ps.tile([128, 512], f32, tag="wps")
            for _ in range(28):
                nc.tensor.matmul(wps[:, :], lhsT=wsrc[:, 0:128], rhs=wsrc[:, :],
                                 start=True, stop=True)
            wout = setup.tile([1, 4], f32, tag="wout")
            nc.vector.tensor_copy(wout[:], wps[0:1, 0:4])

        # ---- inputs to SBUF ----
        xt = []
        for mc in range(6):
            t = setup.tile([128, 2048], bf16, tag=f"xt{mc}", name=f"xt{mc}")
            nc.sync.dma_start(t[:], xt_d[ts(mc, 128), :])
            xt.append(t)

        wqk = []
        for mc in range(6):
            t = setup.tile([128, 384], bf16, tag=f"wqk{mc}", name=f"wqk{mc}")
            nc.sync.dma_start(t[:], wqk_d[ts(mc, 128), :])
            wqk.append(t)

        wv = []
        for mc in range(6):
            t = setup.tile([128, 192], bf16, tag=f"wv{mc}", name=f"wv{mc}")
            nc.sync.dma_start(t[:], wv_d[ts(mc, 128), :])
            wv.append(t)

        wo01 = setup.tile([128, 768], bf16, tag="wo01")
        nc.sync.dma_start(wo01[:], wo_d[0:128, :])
        wo2 = setup.tile([64, 768], bf16, tag="wo2")
        nc.sync.dma_start(wo2[:], wo_d[128:192, :])

        bo_bc = setup.tile([128, 768], f32, tag="bo")
        nc.sync.dma_start(bo_bc[:], bo_d[:, :])
        bqk = setup.tile([128, 3], f32, tag="bqk")
        nc.sync.dma_start(bqk[:], bqk_d[:, :])
        mask = setup.tile([128, 128], bf16, tag="mask")
        nc.sync.dma_start(mask[:], mask_d[:, :])

        # ---- persistent activations ----
        QT01 = setup.tile([128, 2048], bf16, tag="QT01")  # h0 rows 0:64, h1 64:128
        KT01 = setup.tile([128, 2048], bf16, tag="KT01")
        Q2T = setup.tile([64, 2048], bf16, tag="Q2T")
        K2T = setup.tile([64, 2048], bf16, tag="K2T")
        Q2Tb = setup.tile([128, 2048], bf16, tag="Q2Tb")  # rows 64:128 used
        K2Tb = setup.tile([128, 2048], bf16, tag="K2Tb")
        Vp = setup.tile([128, 3120], bf16, tag="Vp")  # [s128, (h, sb, 65)]
        Vp_r = Vp[:].rearrange("p (h s d) -> p h s d", h=3, s=16)
        zT01 = setup.tile([128, 2048], bf16, tag="zT01")
        zT2 = setup.tile([64, 2048], bf16, tag="zT2")
        zs = [setup.tile([65, 2048], f32, tag=f"zs{h}", name=f"zs{h}")
              for h in range(3)]

        kshift = ctx.enter_context(tc.tile_pool(name="kshift", bufs=2))
        nrm = ctx.enter_context(tc.tile_pool(name="nrm", bufs=4))
        nrm_dram = ctx.enter_context(tc.tile_pool(name="nrm_dram", bufs=2, space="DRAM"))

        # ================= projections =================
        with tc.tile_pool(name="proj_ps", bufs=3, space="PSUM") as proj_ps:
            # Q^T/K^T: 3 M-passes: [Qh0|Qh1], [Kh0|Kh1], [Qh2|Kh2]
            for pi, c0 in enumerate((0, 128, 256)):
                for st in range(4):
                    ps = proj_ps.tile([128, 512], f32, tag="qkps")
                    for mc in range(6):
                        nc.tensor.matmul(
                            ps[:, :],
                            lhsT=wqk[mc][:, c0:c0 + 128],
                            rhs=xt[mc][:, ts(st, 512)],
                            start=(mc == 0),
                            stop=(mc == 5),
                        )
                    if pi == 0:
                        nc.vector.tensor_scalar_add(
                            QT01[:, ts(st, 512)], ps[:, :], bqk[:, 0:1])
                    elif pi == 1:
                        nc.vector.tensor_scalar_add(
                            KT01[:, ts(st, 512)], ps[:, :], bqk[:, 1:2])
                    else:
                        nc.scalar.activation(
                            Q2T[:, ts(st, 512)], ps[0:64, :], Ident,
                            bias=bqk[0:64, 2:3])
                        ksh = kshift.tile([128, 512], bf16, tag="ksh",
                                          name=f"ksh{st}")
                        nc.vector.tensor_scalar_add(
                            ksh[64:128, :], ps[64:128, :], bqk[64:128, 2:3])
                        # K2T (base 0) via DMA partition shift; base-64 dups
                        # enable kb-parity pairing for head 2's scores
                        nc.sync.dma_start(K2T[:, ts(st, 512)], ksh[64:128, :])
                        nc.sync.dma_start(K2Tb[64:128, ts(st, 512)], ksh[64:128, :])
                        nc.sync.dma_start(Q2Tb[64:128, ts(st, 512)], Q2T[:, ts(st, 512)])

            # V natural layout; ones column via memset
            nc.vector.memset(Vp_r[:, :, :, 64:65], 1.0)
            for sb in range(16):
                ps = proj_ps.tile([128, 512], f32, tag="vps")
                o = ps[:, 0:192]
                for mc in range(6):
                    nc.tensor.matmul(
                        o,
                        lhsT=xt[mc][:, ts(sb, 128)],
                        rhs=wv[mc][:, :],
                        start=(mc == 0),
                        stop=(mc == 5),
                    )
                nc.vector.tensor_copy(
                    Vp_r[:, :, sb, 0:64],
                    ps[:, 0:192].rearrange("p (h d) -> p h d", h=3),
                )

        # ================= attention (all 3 heads interleaved) =================
        last_staging = [None]

        def normalize(hlist):
            """zT = zs[0:64] / zs[64]; reciprocal via exp(-ln) on ACT, batched
            so the Ln/Exp table sets load once; broadcast via DRAM."""
            n = len(hlist) * 4
            den_all = nrm.tile([n, 512], f32, tag="den_all", name=f"den{hlist[0]}")
            for i, h in enumerate(hlist):
                for qt in range(4):
                    nc.sync.dma_start(
                        den_all[i * 4 + qt:i * 4 + qt + 1, :],
                        zs[h][64:65, ts(qt, 512)],
                    )
            ln_all = nrm.tile([n, 512], f32, tag="ln_all", name=f"ln{hlist[0]}")
            nc.scalar.activation(ln_all[:], den_all[:], Log)
            rec_all = nrm.tile([n, 512], f32, tag="rec_all", name=f"rec{hlist[0]}")
            nc.scalar.activation(rec_all[:], ln_all[:], Exp, scale=-1.0)
            dsc = nrm_dram.tile([n, 512], f32, tag="dsc", name=f"dsc{hlist[0]}")
            nc.sync.dma_start(dsc[:], rec_all[:])
            for i, h in enumerate(hlist):
                for qt in range(4):
                    u = i * 4 + qt
                    rbc = nrm.tile([64, 512], f32, tag="rbc", name=f"rbc{h}_{qt}")
                    nc.sync.dma_start(
                        rbc[:], dsc[u:u + 1, :].broadcast_to((64, 512))
                    )
                    if h == 0:
                        nc.vector.tensor_mul(
                            zT01[0:64, ts(qt, 512)], zs[h][0:64, ts(qt, 512)], rbc[:])
                    elif h == 2:
                        nc.vector.tensor_mul(
                            zT2[:, ts(qt, 512)], zs[h][0:64, ts(qt, 512)], rbc[:])
                    else:
                        # h1 lands on partitions 64:128 of zT01: DVE is
                        # partition-locked, so mul to a temp then DMA-shift
                        zsh = nrm.tile([64, 512], bf16, tag="zsh", name=f"zsh{qt}")
                        nc.vector.tensor_mul(
                            zsh[:], zs[h][0:64, ts(qt, 512)], rbc[:])
                        nc.sync.dma_start(zT01[64:128, ts(qt, 512)], zsh[:])

        with (
            tc.tile_pool(name="sc_ps", bufs=2, space="PSUM") as sc_ps,
            tc.tile_pool(name="zp_ps", bufs=1, space="PSUM") as zp_ps,
            tc.tile_pool(name="expp", bufs=6) as expp,
        ):
            heads = [
                (0, lambda kb, j: KT01[0:64, ts(kb, 128)],
                 lambda q, j, c0: QT01[0:64, q * 512 + c0:(q + 1) * 512]),
                (1, lambda kb, j: KT01[64:128, ts(kb, 128)],
                 lambda q, j, c0: QT01[64:128, q * 512 + c0:(q + 1) * 512]),
                (2, lambda kb, j: K2T[:, ts(kb, 128)] if j == 0
                    else K2Tb[64:128, ts(kb, 128)],
                 lambda q, j, c0: Q2T[:, q * 512 + c0:(q + 1) * 512] if j == 0
                    else Q2Tb[64:128, q * 512 + c0:(q + 1) * 512]),
            ]
            for qt in range(4):
                nkb = 4 * qt + 4
                zp = {}
                for h, _, _ in heads:
                    zp[h] = zp_ps.tile([65, 512], f32, tag=f"zp{h}", name=f"zp{h}")
                for g in range(nkb // 2):
                    kbs = (2 * g, 2 * g + 1)
                    qc0s = [max(0, 128 * (kb - 4 * qt)) for kb in kbs]
                    sc = {}
                    for h, _, _ in heads:
                        sc[h] = sc_ps.tile([128, 1024], f32, tag="sc", name=f"sc{h}")
                    # row-group pairs sit adjacent on PE for concurrency
                    for j, kb in enumerate(kbs):
                        for h, kt_fn, qt_fn in heads:
                            nc.tensor.matmul(
                                sc[h][:, 512 * j + qc0s[j]:512 * (j + 1)],
                                lhsT=kt_fn(kb, j),
                                rhs=qt_fn(qt, j, qc0s[j]),
                                start=True,
                                stop=True,
                            )
                    diag = kbs[0] >= 4 * qt
                    ex = {}
                    for h, _, _ in heads:
                        e = expp.tile([128, 1024], bf16, tag="ex", name=f"ex{h}")
                        if not diag:
                            nc.scalar.activation(e[:], sc[h][:], Exp, scale=SCALE)
                        else:
                            for j in range(2):
                                s0 = 512 * j + qc0s[j]
                                nc.scalar.activation(
                                    e[:, s0:512 * (j + 1)],
                                    sc[h][:, s0:512 * (j + 1)],
                                    Exp, scale=SCALE)
                                nc.vector.tensor_mul(
                                    e[:, s0:s0 + 128], e[:, s0:s0 + 128], mask[:])
                        ex[h] = e
                    for j, kb in enumerate(kbs):
                        for h, _, _ in heads:
                            nc.tensor.matmul(
                                zp[h][:, qc0s[j]:512],
                                lhsT=Vp_r[:, h, kb, :],
                                rhs=ex[h][:, 512 * j + qc0s[j]:512 * (j + 1)],
                                start=(kb == 0),
                                stop=(kb == nkb - 1),
                                skip_group_check=True,
                            )
                for h, _, _ in heads:
                    cp = nc.scalar.copy(zs[h][:, ts(qt, 512)], zp[h][:, :])
                    last_staging[0] = cp

        normalize([0, 1, 2])

        # ================= output projection =================
        with (
            tc.tile_pool(name="out_ps", bufs=3, space="PSUM") as out_ps,
            tc.tile_pool(name="outp", bufs=3) as outp,
            tc.tile_pool(name="warm2_ps", bufs=1, space="PSUM") as warm2_ps,
        ):
            # bf16 warmup pinned behind the last staging copy: keeps the PE
            # clock warm through the normalize tail so outproj starts fast
            wps2 = warm2_ps.tile([128, 512], f32, tag="wps2")
            first = None
            for i in range(14):
                mm = nc.tensor.matmul(wps2[:, :], lhsT=QT01[:, 0:128],
                                      rhs=QT01[:, 0:512], start=True, stop=True)
                if first is None:
                    first = mm
            if last_staging[0] is not None:
                _add_dep_helper(first.ins, last_staging[0].ins, sync=True,
                                reason="pin warmup into normalize tail")
            wout2 = setup.tile([1, 4], f32, tag="wout2")
            nc.vector.tensor_copy(wout2[:], wps2[0:1, 0:4])

            for st in range(16):
                ps = out_ps.tile([128, 768], f32, tag="ops")
                for n0, nw in ((0, 512), (512, 256)):
                    nc.tensor.matmul(
                        ps[:, n0:n0 + nw],
                        lhsT=zT01[:, ts(st, 128)],
                        rhs=wo01[:, n0:n0 + nw],
                        start=True, stop=False,
                    )
                    nc.tensor.matmul(
                        ps[:, n0:n0 + nw],
                        lhsT=zT2[:, ts(st, 128)],
                        rhs=wo2[:, n0:n0 + nw],
                        start=False, stop=True,
                    )
                ot = outp.tile([128, 768], f32, tag="ot")
                nc.vector.tensor_add(ot[:], ps[:, :], bo_bc[:])
                nc.sync.dma_start(out_d[ts(st, 128), :], ot[:])

    nc.compile()
    _CACHE["nc"] = nc
    return nc


def _prep_core_inputs(c, x, W_Q, W_K, W_V, b_Q, b_K, b_V, W_O, b_O):
    b = c // 4
    h0 = HEADS_PER_CORE * (c % 4)
    hs = [h0, h0 + 1, h0 + 2]

    xt = np.empty((769, 2048), np.float32)
    xt[:768] = x[b].T
    xt[768] = 1.0

    def aug(W, bias):  # [768,64]+[64] -> [769,64]
        return np.concatenate([W, bias[None, :]], axis=0)

    wqk = np.concatenate(
        [
            aug(W_Q[hs[0]], b_Q[hs[0]]), aug(W_Q[hs[1]], b_Q[hs[1]]),
            aug(W_K[hs[0]], b_K[hs[0]]), aug(W_K[hs[1]], b_K[hs[1]]),
            aug(W_Q[hs[2]], b_Q[hs[2]]), aug(W_K[hs[2]], b_K[hs[2]]),
        ],
        axis=1,
    )  # [769, 384]
    wv = np.concatenate([W_V[h] for h in hs], axis=1)  # [768, 192]
    wo = np.concatenate([W_O[h] for h in hs], axis=0)  # [192, 768]
    # b_O added once per batch group (4 cores sum); each core's own b_V
    # contribution is exact because attention rows sum to 1.
    bo_eff = b_O / 4.0 + sum(b_V[h] @ W_O[h] for h in hs)
    bo = np.broadcast_to(bo_eff[None, :], (128, 768)).astype(np.float32).copy()
    bqk = np.stack([
        np.concatenate([b_Q[hs[0]], b_Q[hs[1]]]),
        np.concatenate([b_K[hs[0]], b_K[hs[1]]]),
        np.concatenate([b_Q[hs[2]], b_K[hs[2]]]),
    ], axis=1).astype(np.float32)

    kr = np.arange(128)[:, None]
    cc = np.arange(128)[None, :]
    mask = (cc >= kr)  # [128, 128] causal triangle

    return {
        "xt": xt.astype(BF16),
        "mask": mask.astype(BF16),
        "bqk": bqk,
        "wqk": wqk.astype(BF16),
        "wv": wv.astype(BF16),
        "wo": wo.astype(BF16),
        "bo": bo,
    }


def run_sharded(inputs, trace=False, trace_cores=None):
    """Run the SPMD kernel; returns (out [2,2048,768] f32, BassKernelResults)."""
    _ensure_ntff_hook()
    from concourse.bass_utils import run_bass_kernel_spmd

    nc = build_bass()
    in_maps = [
        _prep_core_inputs(c, inputs["normalized_resid_pre"], inputs["W_Q"],
                          inputs["W_K"], inputs["W_V"], inputs["b_Q"], inputs["b_K"],
                          inputs["b_V"], inputs["W_O"], inputs["b_O"])
        for c in range(N_CORES)
    ]
    kwargs = {}
    if trace:
        kwargs["trace"] = True
        kwargs["trace_cores"] = trace_cores if trace_cores is not None else [0]
    res = run_bass_kernel_spmd(nc, in_maps, core_ids=list(range(N_CORES)), **kwargs)

    out = np.zeros((B, S, D_MODEL), np.float32)
    for c in range(N_CORES):
        out[c // 4] += res.results[c]["out"]
    return out, res


def kernel(normalized_resid_pre, W_Q, W_K, W_V, b_Q, b_K, b_V, W_O, b_O):
    inputs = dict(normalized_resid_pre=np.asarray(normalized_resid_pre, np.float32),
                  W_Q=np.asarray(W_Q, np.float32), W_K=np.asarray(W_K, np.float32),
                  W_V=np.asarray(W_V, np.float32), b_Q=np.asarray(b_Q, np.float32),
                  b_K=np.asarray(b_K, np.float32), b_V=np.asarray(b_V, np.float32),
                  W_O=np.asarray(W_O, np.float32), b_O=np.asarray(b_O, np.float32))
    out, _ = run_sharded(inputs, trace=False)
    return out


# revision 24
# speedup vs baseline: 1.2041x; 1.1026x over previous
"""Trainium2 Bass kernel for causal multi-head attention.

Problem shapes (hardcoded):
  x: [B=2, S=2048, D_MODEL=768] f32
  W_Q/W_K/W_V: [H=12, 768, 64], W_O: [12, 64, 768], b_*: per-head biases
  out: [2, 2048, 768] f32

Sharding: 8 cores; core c owns batch b = c // 4 and heads [3*(c%4), 3*(c%4)+3).
Each core computes a partial output over its 3 heads; the host sums the 4
partials per batch (the "all-reduce" of the output projection).

Per-core dataflow (bf16 matmul operands, fp32 PSUM accumulation):
  xT_aug = [x[b].T; ones]                            [769, 2048]  (host)
  Q^T/K^T = W_pair.T @ xT  (heads packed in M)       [128, 2048]  PE
  V       = xT_blk.T @ Wv  (natural layout)          [2048, 3*64] PE
  scores^T[kb, qt] = K^T_blk.T @ Q^T                 [128, 512]   PE (row-
     group pairing: h0 at partitions 0:64, h1 at 64:128 run concurrently;
     head 2 pairs across kb parity via base-64 duplicate tiles)
  expT = exp(0.125 * scores^T)  (diagonal tiles skip fully-masked columns;
     the 128-wide triangle gets a 0/1 bf16 mask multiply)       ACT+DVE
  z'^T += V'_blk.T @ expT   (V' ones column => row 64 = softmax denom) PE
  zT = z'[0:64] * (1/denom)  (1/x = exp(-ln(x)) on ACT, batched;
     broadcast via DRAM round trip)                  DVE
  out[st] = zT01_blk.T @ WO01 + zT2_blk.T @ WO2 + bo  PE + DVE

All biases are exact: b_Q/b_K ride the PSUM->SBUF copies as per-partition
bias APs; b_V/b_O fold into bo on the host (attention rows sum to 1).
"""

import sys
import types

for _p in ("/opt/trn_rl_repo",):
    if _p not in sys.path:
        sys.path.insert(0, _p)

import numpy as np
import ml_dtypes

BF16 = ml_dtypes.bfloat16

B, S, D_MODEL, N_HEADS, D_HEAD = 2, 2048, 768, 12, 64
N_CORES = 8
HEADS_PER_CORE = 3
SCALE = 1.0 / 8.0  # 1/sqrt(d_head)

_CACHE = {}


def _ensure_ntff_hook():
    """Register the axon NTFF profile hook if the image lacks antenv.axon_hooks."""
    try:
        import antenv.axon_hooks  # noqa: F401
        return
    except ImportError:
        pass
    import antenv
    mod = types.ModuleType("antenv.axon_hooks")
    _h = [None]
    mod.set_axon_ntff_profile_hook = lambda h: _h.__setitem__(0, h)
    mod.get_axon_ntff_profile_hook = lambda: _h[0]
    sys.modules["antenv.axon_hooks"] = mod
    antenv.axon_hooks = mod
    try:
        from trn_agent_boot.trn_boot import _ntff_profile_via_ctypes
        hook = _ntff_profile_via_ctypes("/opt/axon/libaxon_pjrt.so")
        if hook is not None:
            mod.set_axon_ntff_profile_hook(hook)
    except Exception:
        pass


def build_bass():
    """Build and compile the per-core Bass program (same NEFF on all 8 cores)."""
    if "nc" in _CACHE:
        return _CACHE["nc"]

    import concourse.mybir as mybir
    import concourse.tile as tile
    from concourse import bacc
    from concourse.bass import ts, _add_dep_helper
    from contextlib import ExitStack

    f32 = mybir.dt.float32
    bf16 = mybir.dt.bfloat16
    Exp = mybir.ActivationFunctionType.Exp
    Log = mybir.ActivationFunctionType.Ln
    Ident = mybir.ActivationFunctionType.Identity

    nc = bacc.Bacc("TRN2", target_bir_lowering=False, debug=False, num_devices=N_CORES)

    xt_d = nc.dram_tensor("xt", [769, 2048], bf16, kind="ExternalInput").ap()
    wqk_d = nc.dram_tensor("wqk", [769, 384], bf16, kind="ExternalInput").ap()
    wv_d = nc.dram_tensor("wv", [768, 192], bf16, kind="ExternalInput").ap()
    wo_d = nc.dram_tensor("wo", [192, 768], bf16, kind="ExternalInput").ap()
    bo_d = nc.dram_tensor("bo", [128, 768], f32, kind="ExternalInput").ap()
    mask_d = nc.dram_tensor("mask", [128, 128], bf16, kind="ExternalInput").ap()
    bqk_d = nc.dram_tensor("bqk", [128, 3], f32, kind="ExternalInput").ap()
    out_d = nc.dram_tensor("out", [2048, 768], f32, kind="ExternalOutput").ap()

    with tile.TileContext(nc) as tc, ExitStack() as ctx:
        setup = ctx.enter_context(tc.tile_pool(name="setup", bufs=1))

        # ---- PE warmup while input DMAs stream in: flips the HAM clock
        # gate to 8/8 (2.4 GHz) before real work ----
        with tc.tile_pool(name="warm_ps", bufs=1, space="PSUM") as warm_ps:
            wsrc = setup.tile([128, 512], bf16, tag="wsrc")
            nc.vector.memset(wsrc[:], 0.0)
            wps = warm_ps.tile([128, 512], f32, tag="wps")
            for _ in range(28):
                nc.tensor.matmul(wps[:, :], lhsT=wsrc[:, 0:128], rhs=wsrc[:, :],
                                 start=True, stop=True)
            wout = setup.tile([1, 4], f32, tag="wout")
            nc.vector.tensor_copy(wout[:], wps[0:1, 0:4])

        # ---- inputs to SBUF ----
        xt = []
        for mc in range(6):
            t = setup.tile([128, 2048], bf16, tag=f"xt{mc}", name=f"xt{mc}")
            nc.sync.dma_start(t[:], xt_d[ts(mc, 128), :])
            xt.append(t)

        wqk = []
        for mc in range(6):
            t = setup.tile([128, 384], bf16, tag=f"wqk{mc}", name=f"wqk{mc}")
            nc.sync.dma_start(t[:], wqk_d[ts(mc, 128), :])
            wqk.append(t)

        wv = []
        for mc in range(6):
            t = setup.tile([128, 192], bf16, tag=f"wv{mc}", name=f"wv{mc}")
            nc.sync.dma_start(t[:], wv_d[ts(mc, 128), :])
            wv.append(t)

        wo01 = setup.tile([128, 768], bf16, tag="wo01")
        nc.sync.dma_start(wo01[:], wo_d[0:128, :])
        wo2 = setup.tile([64, 768], bf16, tag="wo2")
        nc.sync.dma_start(wo2[:], wo_d[128:192, :])

        bo_bc = setup.tile([128, 768], f32, tag="bo")
        nc.sync.dma_start(bo_bc[:], bo_d[:, :])
        bqk = setup.tile([128, 3], f32, tag="bqk")
        nc.sync.dma_start(bqk[:], bqk_d[:, :])
        mask = setup.tile([128, 128], bf16, tag="mask")
        nc.sync.dma_start(mask[:], mask_d[:, :])

        # ---- persistent activations ----
        QT01 = setup.tile([128, 2048], bf16, tag="QT01")  # h0 rows 0:64, h1 64:128
        KT01 = setup.tile([128, 2048], bf16, tag="KT01")
        Q2T = setup.tile([64, 2048], bf16, tag="Q2T")
        K2T = setup.tile([64, 2048], bf16, tag="K2T")
        Q2Tb = setup.tile([128, 2048], bf16, tag="Q2Tb")  # rows 64:128 used
        K2Tb = setup.tile([128, 2048], bf16, tag="K2Tb")
        Vp = setup.tile([128, 3120], bf16, tag="Vp")  # [s128, (h, sb, 65)]
        Vp_r = Vp[:].rearrange("p (h s d) -> p h s d", h=3, s=16)
        zT01 = setup.tile([128, 2048], bf16, tag="zT01")
        zT2 = setup.tile([64, 2048], bf16, tag="zT2")
        zs = [setup.tile([65, 2048], f32, tag=f"zs{h}", name=f"zs{h}")
              for h in range(3)]

        kshift = ctx.enter_context(tc.tile_pool(name="kshift", bufs=2))
        nrm = ctx.enter_context(tc.tile_pool(name="nrm", bufs=4))
        nrm_dram = ctx.enter_context(tc.tile_pool(name="nrm_dram", bufs=2, space="DRAM"))

        # ================= projections =================
        with tc.tile_pool(name="proj_ps", bufs=3, space="PSUM") as proj_ps:
            # Q^T/K^T: 3 M-passes: [Qh0|Qh1], [Kh0|Kh1], [Qh2|Kh2]
            for pi, c0 in enumerate((0, 128, 256)):
                for st in range(4):
                    ps = proj_ps.tile([128, 512], f32, tag="qkps")
                    for mc in range(6):
                        nc.tensor.matmul(
                            ps[:, :],
                            lhsT=wqk[mc][:, c0:c0 + 128],
                            rhs=xt[mc][:, ts(st, 512)],
                            start=(mc == 0),
                            stop=(mc == 5),
                        )
                    if pi == 0:
                        nc.vector.tensor_scalar_add(
                            QT01[:, ts(st, 512)], ps[:, :], bqk[:, 0:1])
                    elif pi == 1:
                        nc.vector.tensor_scalar_add(
                            KT01[:, ts(st, 512)], ps[:, :], bqk[:, 1:2])
                    else:
                        nc.scalar.activation(
                            Q2T[:, ts(st, 512)], ps[0:64, :], Ident,
                            bias=bqk[0:64, 2:3])
                        ksh = kshift.tile([128, 512], bf16, tag="ksh",
                                          name=f"ksh{st}")
                        nc.vector.tensor_scalar_add(
                            ksh[64:128, :], ps[64:128, :], bqk[64:128, 2:3])
                        # K2T (base 0) via DMA partition shift; base-64 dups
                        # enable kb-parity pairing for head 2's scores
                        nc.sync.dma_start(K2T[:, ts(st, 512)], ksh[64:128, :])
                        nc.sync.dma_start(K2Tb[64:128, ts(st, 512)], ksh[64:128, :])
                        nc.sync.dma_start(Q2Tb[64:128, ts(st, 512)], Q2T[:, ts(st, 512)])

            # V natural layout; ones column via memset
            nc.vector.memset(Vp_r[:, :, :, 64:65], 1.0)
            for sb in range(16):
                ps = proj_ps.tile([128, 512], f32, tag="vps")
                o = ps[:, 0:192]
                for mc in range(6):
                    nc.tensor.matmul(
                        o,
                        lhsT=xt[mc][:, ts(sb, 128)],
                        rhs=wv[mc][:, :],
                        start=(mc == 0),
                        stop=(mc == 5),
                    )
                nc.vector.tensor_copy(
                    Vp_r[:, :, sb, 0:64],
                    ps[:, 0:192].rearrange("p (h d) -> p h d", h=3),
                )

        # ================= attention (all 3 heads interleaved) =================
        last_staging = [None]

        def normalize(hlist):
            """zT = zs[0:64] / zs[64]; reciprocal via exp(-ln) on ACT, batched
            so the Ln/Exp table sets load once; broadcast via DRAM."""
            n = len(hlist) * 4
            den_all = nrm.tile([n, 512], f32, tag="den_all", name=f"den{hlist[0]}")
            for i, h in enumerate(hlist):
                for qt in range(4):
                    nc.sync.dma_start(
                        den_all[i * 4 + qt:i * 4 + qt + 1, :],
                        zs[h][64:65, ts(qt, 512)],
                    )
            ln_all = nrm.tile([n, 512], f32, tag="ln_all", name=f"ln{hlist[0]}")
            nc.scalar.activation(ln_all[:], den_all[:], Log)
            rec_all = nrm.tile([n, 512], f32, tag="rec_all", name=f"rec{hlist[0]}")
            nc.scalar.activation(rec_all[:], ln_all[:], Exp, scale=-1.0)
            dsc = nrm_dram.tile([n, 512], f32, tag="dsc", name=f"dsc{hlist[0]}")
            nc.sync.dma_start(dsc[:], rec_all[:])
            for i, h in enumerate(hlist):
                for qt in range(4):
                    u = i * 4 + qt
                    rbc = nrm.tile([64, 512], f32, tag="rbc", name=f"rbc{h}_{qt}")
                    nc.sync.dma_start(
                        rbc[:], dsc[u:u + 1, :].broadcast_to((64, 512))
                    )
                    if h == 0:
                        nc.vector.tensor_mul(
                            zT01[0:64, ts(qt, 512)], zs[h][0:64, ts(qt, 512)], rbc[:])
                    elif h == 2:
                        nc.vector.tensor_mul(
                            zT2[:, ts(qt, 512)], zs[h][0:64, ts(qt, 512)], rbc[:])
                    else:
                        # h1 lands on partitions 64:128 of zT01: DVE is
                        # partition-locked, so mul to a temp then DMA-shift
                        zsh = nrm.tile([64, 512], bf16, tag="zsh", name=f"zsh{qt}")
                        nc.vector.tensor_mul(
                            zsh[:], zs[h][0:64, ts(qt, 512)], rbc[:])
                        nc.sync.dma_start(zT01[64:128, ts(qt, 512)], zsh[:])

        with (
            tc.tile_pool(name="sc_ps", bufs=3, space="PSUM") as sc_ps,
            tc.tile_pool(name="zp_ps", bufs=1, space="PSUM") as zp_ps,
            tc.tile_pool(name="expp", bufs=6) as expp,
        ):
            def attend(heads):
                for qt in range(4):
                    nkb = 4 * qt + 4
                    zp = {}
                    for h, _, _ in heads:
                        zp[h] = zp_ps.tile([65, 512], f32, tag=f"zp{h % 2}", name=f"zp{h}")
                    for g in range(nkb // 2):
                        kbs = (2 * g, 2 * g + 1)
                        qc0s = [max(0, 128 * (kb - 4 * qt)) for kb in kbs]
                        sc = {}
                        for h, _, _ in heads:
                            sc[h] = sc_ps.tile([128, 1024], f32, tag="sc", name=f"sc{h}")
                        # row-group pairs sit adjacent on PE for concurrency
                        for j, kb in enumerate(kbs):
                            for h, kt_fn, qt_fn in heads:
                                nc.tensor.matmul(
                                    sc[h][:, 512 * j + qc0s[j]:512 * (j + 1)],
                                    lhsT=kt_fn(kb, j),
                                    rhs=qt_fn(qt, j, qc0s[j]),
                                    start=True,
                                    stop=True,
                                )
                        diag = kbs[0] >= 4 * qt
                        ex = {}
                        for h, _, _ in heads:
                            e = expp.tile([128, 1024], bf16, tag="ex", name=f"ex{h}")
                            if not diag:
                                nc.scalar.activation(e[:], sc[h][:], Exp, scale=SCALE)
                            else:
                                for j in range(2):
                                    s0 = 512 * j + qc0s[j]
                                    nc.scalar.activation(
                                        e[:, s0:512 * (j + 1)],
                                        sc[h][:, s0:512 * (j + 1)],
                                        Exp, scale=SCALE)
                                    nc.vector.tensor_mul(
                                        e[:, s0:s0 + 128], e[:, s0:s0 + 128], mask[:])
                            ex[h] = e
                        for j, kb in enumerate(kbs):
                            for h, _, _ in heads:
                                nc.tensor.matmul(
                                    zp[h][:, qc0s[j]:512],
                                    lhsT=Vp_r[:, h, kb, :],
                                    rhs=ex[h][:, 512 * j + qc0s[j]:512 * (j + 1)],
                                    start=(kb == 0),
                                    stop=(kb == nkb - 1),
                                    skip_group_check=True,
                                )
                    for h, _, _ in heads:
                        cp = nc.scalar.copy(zs[h][:, ts(qt, 512)], zp[h][:, :])
                        last_staging[0] = cp

            # pass A: heads 0,1 row-packed (concurrent PE row groups)
            attend([
                (0, lambda kb, j: KT01[0:64, ts(kb, 128)],
                 lambda q, j, c0: QT01[0:64, q * 512 + c0:(q + 1) * 512]),
                (1, lambda kb, j: KT01[64:128, ts(kb, 128)],
                 lambda q, j, c0: QT01[64:128, q * 512 + c0:(q + 1) * 512]),
            ])
            normalize([0, 1])  # overlaps pass B on PE
            # pass B: head 2, kb-parity pairing via base-64 duplicates
            attend([
                (2, lambda kb, j: K2T[:, ts(kb, 128)] if j == 0
                    else K2Tb[64:128, ts(kb, 128)],
                 lambda q, j, c0: Q2T[:, q * 512 + c0:(q + 1) * 512] if j == 0
                    else Q2Tb[64:128, q * 512 + c0:(q + 1) * 512]),
            ])

        normalize([2])


        # ================= output projection =================
        with (
            tc.tile_pool(name="out_ps", bufs=3, space="PSUM") as out_ps,
            tc.tile_pool(name="outp", bufs=3) as outp,
            tc.tile_pool(name="warm2_ps", bufs=1, space="PSUM") as warm2_ps,
        ):
            # bf16 warmup pinned behind the last staging copy: keeps the PE
            # clock warm through the normalize tail so outproj starts fast
            wps2 = warm2_ps.tile([128, 512], f32, tag="wps2")
            first = None
            for i in range(26):
                mm = nc.tensor.matmul(wps2[:, :], lhsT=QT01[:, 0:128],
                                      rhs=QT01[:, 0:512], start=True, stop=True)
                if first is None:
                    first = mm
            if last_staging[0] is not None:
                _add_dep_helper(first.ins, last_staging[0].ins, sync=True,
                                reason="pin warmup into normalize tail")
            wout2 = setup.tile([1, 4], f32, tag="wout2")
            nc.vector.tensor_copy(wout2[:], wps2[0:1, 0:4])

            for st in range(16):
                ps = out_ps.tile([128, 768], f32, tag="ops")
                for n0, nw in ((0, 512), (512, 256)):
                    nc.tensor.matmul(
                        ps[:, n0:n0 + nw],
                        lhsT=zT01[:, ts(st, 128)],
                        rhs=wo01[:, n0:n0 + nw],
                        start=True, stop=False,
                    )
                    nc.tensor.matmul(
                        ps[:, n0:n0 + nw],
                        lhsT=zT2[:, ts(st, 128)],
                        rhs=wo2[:, n0:n0 + nw],
                        start=False, stop=True,
                    )
                ot = outp.tile([128, 768], f32, tag="ot")
                nc.vector.tensor_add(ot[:], ps[:, :], bo_bc[:])
                nc.sync.dma_start(out_d[ts(st, 128), :], ot[:])

    nc.compile()
    _CACHE["nc"] = nc
    return nc


def _prep_core_inputs(c, x, W_Q, W_K, W_V, b_Q, b_K, b_V, W_O, b_O):
    b = c // 4
    h0 = HEADS_PER_CORE * (c % 4)
    hs = [h0, h0 + 1, h0 + 2]

    xt = np.empty((769, 2048), np.float32)
    xt[:768] = x[b].T
    xt[768] = 1.0

    def aug(W, bias):  # [768,64]+[64] -> [769,64]
        return np.concatenate([W, bias[None, :]], axis=0)

    wqk = np.concatenate(
        [
            aug(W_Q[hs[0]], b_Q[hs[0]]), aug(W_Q[hs[1]], b_Q[hs[1]]),
            aug(W_K[hs[0]], b_K[hs[0]]), aug(W_K[hs[1]], b_K[hs[1]]),
            aug(W_Q[hs[2]], b_Q[hs[2]]), aug(W_K[hs[2]], b_K[hs[2]]),
        ],
        axis=1,
    )  # [769, 384]
    wv = np.concatenate([W_V[h] for h in hs], axis=1)  # [768, 192]
    wo = np.concatenate([W_O[h] for h in hs], axis=0)  # [192, 768]
    # b_O added once per batch group (4 cores sum); each core's own b_V
    # contribution is exact because attention rows sum to 1.
    bo_eff = b_O / 4.0 + sum(b_V[h] @ W_O[h] for h in hs)
    bo = np.broadcast_to(bo_eff[None, :], (128, 768)).astype(np.float32).copy()
    bqk = np.stack([
        np.concatenate([b_Q[hs[0]], b_Q[hs[1]]]),
        np.concatenate([b_K[hs[0]], b_K[hs[1]]]),
        np.concatenate([b_Q[hs[2]], b_K[hs[2]]]),
    ], axis=1).astype(np.float32)

    kr = np.arange(128)[:, None]
    cc = np.arange(128)[None, :]
    mask = (cc >= kr)  # [128, 128] causal triangle

    return {
        "xt": xt.astype(BF16),
        "mask": mask.astype(BF16),
        "bqk": bqk,
        "wqk": wqk.astype(BF16),
        "wv": wv.astype(BF16),
        "wo": wo.astype(BF16),
        "bo": bo,
    }


def run_sharded(inputs, trace=False, trace_cores=None):
    """Run the SPMD kernel; returns (out [2,2048,768] f32, BassKernelResults)."""
    _ensure_ntff_hook()
    from concourse.bass_utils import run_bass_kernel_spmd

    nc = build_bass()
    in_maps = [
        _prep_core_inputs(c, inputs["normalized_resid_pre"], inputs["W_Q"],
                          inputs["W_K"], inputs["W_V"], inputs["b_Q"], inputs["b_K"],
                          inputs["b_V"], inputs["W_O"], inputs["b_O"])
        for c in range(N_CORES)
    ]
    kwargs = {}
    if trace:
        kwargs["trace"] = True
        kwargs["trace_cores"] = trace_cores if trace_cores is not None else [0]
    res = run_bass_kernel_spmd(nc, in_maps, core_ids=list(range(N_CORES)), **kwargs)

    out = np.zeros((B, S, D_MODEL), np.float32)
    for c in range(N_CORES):
        out[c // 4] += res.results[c]["out"]
    return out, res


def kernel(normalized_resid_pre, W_Q, W_K, W_V, b_Q, b_K, b_V, W_O, b_O):
    inputs = dict(normalized_resid_pre=np.asarray(normalized_resid_pre, np.float32),
                  W_Q=np.asarray(W_Q, np.float32), W_K=np.asarray(W_K, np.float32),
                  W_V=np.asarray(W_V, np.float32), b_Q=np.asarray(b_Q, np.float32),
                  b_K=np.asarray(b_K, np.float32), b_V=np.asarray(b_V, np.float32),
                  W_O=np.asarray(W_O, np.float32), b_O=np.asarray(b_O, np.float32))
    out, _ = run_sharded(inputs, trace=False)
    return out
